# revision 11
# baseline (speedup 1.0000x reference)
"""Trainium2 Bass kernel for gather + segment-mean aggregation.

Strategy: shard the 500K output segments across 8 NeuronCores (62500 each).
Each core owns the contiguous block of inputs whose (sorted) segment_ids fall
in its range, gathers the referenced source rows with indirect DMA, and
reduces them into per-segment means with a one-hot matmul over 128-segment
windows. Output is gathered by simple concatenation.

Self-contained: hardcodes the problem shapes (1M x 64 source, 4M inputs,
500K segments) and compiles/executes on NeuronCores 0-7 via
concourse/bass_utils.run_bass_kernel_spmd.
"""
import contextlib
import numpy as np

import concourse.bass as bass
import concourse.tile as tile
from concourse import bacc, mybir
from concourse.bass_utils import run_bass_kernel_spmd

NUM_SOURCES = 1_000_000
TOTAL_INPUTS = 4_000_000
NUM_SEGMENTS = 500_000
DIM = 64
N_CORES = 8
SEGS_PER_CORE = NUM_SEGMENTS // N_CORES  # 62500
WIN = 128                                # segments per matmul window
P = 128                                  # inputs per tile (partition dim)

_compiled_cache = {}


def _plan_core(gather_idx, segment_ids, lo_seg, hi_seg):
    """Build the padded tile plan for one core.

    Returns (gidx [P, T], seg_local [P, T], tile_window [T], inv [WIN, n_win]).
    Inputs are already sorted by segment id. Each tile of P inputs is padded
    so it references segments of exactly one WIN-segment window.
    """
    lo = np.searchsorted(segment_ids, lo_seg, side="left")
    hi = np.searchsorted(segment_ids, hi_seg, side="left")
    gi = gather_idx[lo:hi].astype(np.int32)
    si = (segment_ids[lo:hi] - lo_seg).astype(np.int32)
    n_seg = hi_seg - lo_seg
    n_win = (n_seg + WIN - 1) // WIN

    win_of_input = si // WIN
    # number of inputs per window
    counts_w = np.bincount(win_of_input, minlength=n_win)
    tiles_w = np.maximum((counts_w + P - 1) // P, 1)  # >=1 tile per window
    T = int(tiles_w.sum())

    gidx = np.zeros((T * P,), dtype=np.int32)
    segl = np.full((T * P,), -1.0, dtype=np.float32)  # -1 -> zero one-hot row
    tile_window = np.repeat(np.arange(n_win), tiles_w)

    tile_start = np.concatenate([[0], np.cumsum(tiles_w)]) * P
    in_start = np.concatenate([[0], np.cumsum(counts_w)])
    for w in range(n_win):
        n = counts_w[w]
        dst = tile_start[w]
        src = in_start[w]
        gidx[dst:dst + n] = gi[src:src + n]
        segl[dst:dst + n] = (si[src:src + n] - w * WIN).astype(np.float32)

    counts_s = np.bincount(si, minlength=n_win * WIN).astype(np.float32)
    inv = (1.0 / np.maximum(counts_s, 1.0)).reshape(n_win, WIN).T.copy()  # [WIN, n_win]

    # partition-major layout: input (t, p) at flat t*P + p
    gidx = gidx.reshape(T, P).T.copy()       # [P, T]
    segl = segl.reshape(T, P).T.copy()       # [P, T]
    return gidx, segl, tile_window, inv, T, n_win


def _build_program(T, n_win, tile_window):
    """Build and compile the bass program for one core-shape."""
    nc = bacc.Bacc("TRN2", target_bir_lowering=False, debug=False,
                   num_devices=N_CORES)
    src = nc.dram_tensor("src", [NUM_SOURCES, DIM], mybir.dt.float32,
                         kind="ExternalInput").ap()
    gidx = nc.dram_tensor("gidx", [P, T], mybir.dt.int32,
                          kind="ExternalInput").ap()
    segl = nc.dram_tensor("segl", [P, T], mybir.dt.float32,
                          kind="ExternalInput").ap()
    invc = nc.dram_tensor("invc", [WIN, n_win], mybir.dt.float32,
                          kind="ExternalInput").ap()
    iota = nc.dram_tensor("iota", [P, WIN], mybir.dt.float32,
                          kind="ExternalInput").ap()
    out = nc.dram_tensor("out", [P, n_win, DIM], mybir.dt.float32,
                         kind="ExternalOutput").ap()

    OUTB = 8  # windows per staged output DMA

    with tile.TileContext(nc) as tc:
        with contextlib.ExitStack() as ctx:
            const_p = ctx.enter_context(tc.tile_pool(name="const", bufs=1))
            gp = ctx.enter_context(tc.tile_pool(name="g", bufs=32))
            ohp = ctx.enter_context(tc.tile_pool(name="oh", bufs=8))
            pp = ctx.enter_context(tc.tile_pool(name="ps", bufs=8, space="PSUM"))
            op = ctx.enter_context(tc.tile_pool(name="ostage", bufs=3))

            gidx_sb = const_p.tile([P, T], mybir.dt.int32)
            nc.sync.dma_start(gidx_sb[:], gidx[:])
            segl_sb = const_p.tile([P, T], mybir.dt.float32)
            nc.sync.dma_start(segl_sb[:], segl[:])
            inv_sb = const_p.tile([WIN, n_win], mybir.dt.float32)
            nc.sync.dma_start(inv_sb[:], invc[:])
            iota_sb = const_p.tile([P, WIN], mybir.dt.float32)
            nc.sync.dma_start(iota_sb[:], iota[:])

            stage = None
            psum = None
            t = 0
            for w in range(n_win):
                if w % OUTB == 0:
                    stage = op.tile([P, OUTB * DIM], mybir.dt.float32, tag="st")
                first = True
                while t < T and tile_window[t] == w:
                    g = gp.tile([P, DIM], mybir.dt.float32, tag="g")
                    nc.gpsimd.indirect_dma_start(
                        out=g[:], out_offset=None, in_=src[:],
                        in_offset=bass.IndirectOffsetOnAxis(
                            ap=gidx_sb[:, t:t + 1], axis=0))
                    oh = ohp.tile([P, WIN], mybir.dt.float32, tag="oh")
                    nc.vector.tensor_tensor(
                        out=oh[:],
                        in0=segl_sb[:, t:t + 1].to_broadcast([P, WIN]),
                        in1=iota_sb[:],
                        op=mybir.AluOpType.is_equal)
                    if first:
                        psum = pp.tile([WIN, DIM], mybir.dt.float32,
                                       space="PSUM", tag="ps")
                    last = (t + 1 >= T) or (tile_window[t + 1] != w)
                    nc.tensor.matmul(out=psum[:], lhsT=oh[:], rhs=g[:],
                                     start=first, stop=last)
                    first = False
                    t += 1
                # scale by 1/count and stage
                nc.vector.tensor_tensor(
                    out=stage[:, (w % OUTB) * DIM:(w % OUTB + 1) * DIM],
                    in0=psum[:],
                    in1=inv_sb[:, w:w + 1].to_broadcast([WIN, DIM]),
                    op=mybir.AluOpType.mult)
                if w % OUTB == OUTB - 1 or w == n_win - 1:
                    w0 = (w // OUTB) * OUTB
                    nb = w - w0 + 1
                    nc.sync.dma_start(out[:, w0:w0 + nb, :],
                                      stage[:, 0:nb * DIM])
    nc.compile()
    return nc


def kernel(source, gather_idx, segment_ids, num_segments):
    source = np.asarray(source, dtype=np.float32)
    gather_idx = np.asarray(gather_idx)
    segment_ids = np.asarray(segment_ids)
    assert source.shape == (NUM_SOURCES, DIM)
    assert int(num_segments) == NUM_SEGMENTS

    iota_np = np.broadcast_to(
        np.arange(WIN, dtype=np.float32)[None, :], (P, WIN)).copy()

    plans = []
    for c in range(N_CORES):
        lo_seg = c * SEGS_PER_CORE
        hi_seg = (c + 1) * SEGS_PER_CORE
        plans.append(_plan_core(gather_idx, segment_ids, lo_seg, hi_seg))

    # All cores share one program (SPMD): pad every core to the max tile
    # count and use a shared tile->window map. Padding tiles reference
    # window n_win-1 ... simpler: pad each core's plan to a COMMON
    # (T, tile_window) by appending empty tiles to the last window.
    Tmax = max(p[4] for p in plans)
    n_win = plans[0][5]
    assert all(p[5] == n_win for p in plans)

    # common tile_window: to keep one program for all cores, pad each core's
    # per-window tile counts up to the max across cores.
    tiles_per_win = np.zeros((N_CORES, n_win), dtype=np.int64)
    for c, p in enumerate(plans):
        tw = p[2]
        tiles_per_win[c] = np.bincount(tw, minlength=n_win)
    common_tiles_w = tiles_per_win.max(axis=0)
    T = int(common_tiles_w.sum())
    common_tile_window = np.repeat(np.arange(n_win), common_tiles_w)

    in_maps = []
    for c, p in enumerate(plans):
        gidx_c, segl_c, tw_c, inv_c, T_c, _ = p
        # re-pack this core's tiles into the common layout
        gidx_full = np.zeros((P, T), dtype=np.int32)
        segl_full = np.full((P, T), -1.0, dtype=np.float32)
        src_pos = np.concatenate([[0], np.cumsum(tiles_per_win[c])])
        dst_pos = np.concatenate([[0], np.cumsum(common_tiles_w)])
        for w in range(n_win):
            k = tiles_per_win[c][w]
            gidx_full[:, dst_pos[w]:dst_pos[w] + k] = \
                gidx_c[:, src_pos[w]:src_pos[w] + k]
            segl_full[:, dst_pos[w]:dst_pos[w] + k] = \
                segl_c[:, src_pos[w]:src_pos[w] + k]
        in_maps.append({
            "src": source,
            "gidx": gidx_full,
            "segl": segl_full,
            "invc": inv_c,
            "iota": iota_np,
        })

    key = (T, n_win, common_tile_window.tobytes())
    if key not in _compiled_cache:
        _compiled_cache[key] = _build_program(T, n_win, common_tile_window)
    nc = _compiled_cache[key]

    _last_state["nc"] = nc
    _last_state["in_maps"] = in_maps
    try:
        res = run_bass_kernel_spmd(nc, in_maps, core_ids=list(range(N_CORES)))
    except Exception:
        # transient NRT device wedge (NRT_EXEC_UNIT_UNRECOVERABLE) -- retry once
        res = run_bass_kernel_spmd(nc, in_maps, core_ids=list(range(N_CORES)))
    outs = [res.results[c]["out"].transpose(1, 0, 2).reshape(n_win * WIN, DIM)
            [:SEGS_PER_CORE] for c in range(N_CORES)]
    return np.concatenate(outs, axis=0)


# state of the last kernel() invocation, for external profiling harnesses
_last_state = {}


# revision 12
# speedup vs baseline: 1.0502x; 1.0502x over previous
"""Trainium2 Bass kernel for gather + segment-mean aggregation.

Strategy: shard the 500K output segments across 8 NeuronCores (62500 each).
Each core owns the contiguous block of inputs whose (sorted) segment_ids fall
in its range, gathers the referenced source rows with indirect DMA, and
reduces them into per-segment means with a one-hot matmul over 128-segment
windows. Output is gathered by simple concatenation.

Self-contained: hardcodes the problem shapes (1M x 64 source, 4M inputs,
500K segments) and compiles/executes on NeuronCores 0-7 via
concourse/bass_utils.run_bass_kernel_spmd.
"""
import contextlib
import numpy as np

import concourse.bass as bass
import concourse.tile as tile
from concourse import bacc, mybir
from concourse.bass_utils import run_bass_kernel_spmd

NUM_SOURCES = 1_000_000
TOTAL_INPUTS = 4_000_000
NUM_SEGMENTS = 500_000
DIM = 64
N_CORES = 8
SEGS_PER_CORE = NUM_SEGMENTS // N_CORES  # 62500
WIN = 128                                # segments per matmul window
P = 128                                  # inputs per tile (partition dim)

_compiled_cache = {}


def _plan_core(gather_idx, segment_ids, lo_seg, hi_seg):
    """Build the padded tile plan for one core.

    Returns (gidx [P, T], seg_local [P, T], tile_window [T], inv [WIN, n_win]).
    Inputs are already sorted by segment id. Each tile of P inputs is padded
    so it references segments of exactly one WIN-segment window.
    """
    lo = np.searchsorted(segment_ids, lo_seg, side="left")
    hi = np.searchsorted(segment_ids, hi_seg, side="left")
    gi = gather_idx[lo:hi].astype(np.int32)
    si = (segment_ids[lo:hi] - lo_seg).astype(np.int32)
    n_seg = hi_seg - lo_seg
    n_win = (n_seg + WIN - 1) // WIN

    win_of_input = si // WIN
    # number of inputs per window
    counts_w = np.bincount(win_of_input, minlength=n_win)
    tiles_w = np.maximum((counts_w + P - 1) // P, 1)  # >=1 tile per window
    T = int(tiles_w.sum())

    gidx = np.zeros((T * P,), dtype=np.int32)
    segl = np.full((T * P,), -1.0, dtype=np.float32)  # -1 -> zero one-hot row
    tile_window = np.repeat(np.arange(n_win), tiles_w)

    tile_start = np.concatenate([[0], np.cumsum(tiles_w)]) * P
    in_start = np.concatenate([[0], np.cumsum(counts_w)])
    for w in range(n_win):
        n = counts_w[w]
        dst = tile_start[w]
        src = in_start[w]
        gidx[dst:dst + n] = gi[src:src + n]
        segl[dst:dst + n] = (si[src:src + n] - w * WIN).astype(np.float32)

    counts_s = np.bincount(si, minlength=n_win * WIN).astype(np.float32)
    inv = (1.0 / np.maximum(counts_s, 1.0)).reshape(n_win, WIN).T.copy()  # [WIN, n_win]

    # partition-major layout: input (t, p) at flat t*P + p
    gidx = gidx.reshape(T, P).T.copy()       # [P, T]
    segl = segl.reshape(T, P).T.copy()       # [P, T]
    return gidx, segl, tile_window, inv, T, n_win


def _build_program(T, n_win, tile_window):
    """Build and compile the bass program for one core-shape."""
    nc = bacc.Bacc("TRN2", target_bir_lowering=False, debug=False,
                   num_devices=N_CORES)
    src = nc.dram_tensor("src", [NUM_SOURCES, DIM], mybir.dt.float32,
                         kind="ExternalInput").ap()
    gidx = nc.dram_tensor("gidx", [P, T], mybir.dt.int32,
                          kind="ExternalInput").ap()
    segl = nc.dram_tensor("segl", [P, T], mybir.dt.float32,
                          kind="ExternalInput").ap()
    invc = nc.dram_tensor("invc", [WIN, n_win], mybir.dt.float32,
                          kind="ExternalInput").ap()
    iota = nc.dram_tensor("iota", [P, WIN], mybir.dt.float32,
                          kind="ExternalInput").ap()
    out = nc.dram_tensor("out", [P, n_win, DIM], mybir.dt.float32,
                         kind="ExternalOutput").ap()

    OUTB = 8  # windows per staged output DMA

    with tile.TileContext(nc) as tc:
        with contextlib.ExitStack() as ctx:
            const_p = ctx.enter_context(tc.tile_pool(name="const", bufs=1))
            gp = ctx.enter_context(tc.tile_pool(name="g", bufs=32))
            ohp = ctx.enter_context(tc.tile_pool(name="oh", bufs=8))
            pp = ctx.enter_context(tc.tile_pool(name="ps", bufs=8, space="PSUM"))
            op = ctx.enter_context(tc.tile_pool(name="ostage", bufs=3))

            gidx_sb = const_p.tile([P, T], mybir.dt.int32)
            nc.sync.dma_start(gidx_sb[:], gidx[:])
            segl_sb = const_p.tile([P, T], mybir.dt.float32)
            nc.sync.dma_start(segl_sb[:], segl[:])
            inv_sb = const_p.tile([WIN, n_win], mybir.dt.float32)
            nc.sync.dma_start(inv_sb[:], invc[:])
            iota_sb = const_p.tile([P, WIN], mybir.dt.float32)
            nc.sync.dma_start(iota_sb[:], iota[:])

            stage = None
            psum = None
            t = 0
            for w in range(n_win):
                if w % OUTB == 0:
                    stage = op.tile([P, OUTB * DIM], mybir.dt.float32, tag="st")
                first = True
                while t < T and tile_window[t] == w:
                    g = gp.tile([P, DIM], mybir.dt.float32, tag="g")
                    nc.gpsimd.indirect_dma_start(
                        out=g[:], out_offset=None, in_=src[:],
                        in_offset=bass.IndirectOffsetOnAxis(
                            ap=gidx_sb[:, t:t + 1], axis=0))
                    oh = ohp.tile([P, WIN], mybir.dt.float32, tag="oh")
                    nc.vector.tensor_tensor(
                        out=oh[:],
                        in0=segl_sb[:, t:t + 1].to_broadcast([P, WIN]),
                        in1=iota_sb[:],
                        op=mybir.AluOpType.is_equal)
                    if first:
                        psum = pp.tile([WIN, DIM], mybir.dt.float32,
                                       space="PSUM", tag="ps")
                    last = (t + 1 >= T) or (tile_window[t + 1] != w)
                    nc.tensor.matmul(out=psum[:], lhsT=oh[:], rhs=g[:],
                                     start=first, stop=last)
                    first = False
                    t += 1
                # scale by 1/count and stage
                nc.vector.tensor_tensor(
                    out=stage[:, (w % OUTB) * DIM:(w % OUTB + 1) * DIM],
                    in0=psum[:],
                    in1=inv_sb[:, w:w + 1].to_broadcast([WIN, DIM]),
                    op=mybir.AluOpType.mult)
                if w % OUTB == OUTB - 1 or w == n_win - 1:
                    w0 = (w // OUTB) * OUTB
                    nb = w - w0 + 1
                    nc.sync.dma_start(out[:, w0:w0 + nb, :],
                                      stage[:, 0:nb * DIM])
    nc.compile()
    return nc


def kernel(source, gather_idx, segment_ids, num_segments):
    source = np.asarray(source, dtype=np.float32)
    gather_idx = np.asarray(gather_idx)
    segment_ids = np.asarray(segment_ids)
    assert source.shape == (NUM_SOURCES, DIM)
    assert int(num_segments) == NUM_SEGMENTS

    iota_np = np.broadcast_to(
        np.arange(WIN, dtype=np.float32)[None, :], (P, WIN)).copy()

    plans = []
    for c in range(N_CORES):
        lo_seg = c * SEGS_PER_CORE
        hi_seg = (c + 1) * SEGS_PER_CORE
        plans.append(_plan_core(gather_idx, segment_ids, lo_seg, hi_seg))

    # All cores share one program (SPMD): pad every core to the max tile
    # count and use a shared tile->window map. Padding tiles reference
    # window n_win-1 ... simpler: pad each core's plan to a COMMON
    # (T, tile_window) by appending empty tiles to the last window.
    Tmax = max(p[4] for p in plans)
    n_win = plans[0][5]
    assert all(p[5] == n_win for p in plans)

    # common tile_window: to keep one program for all cores, pad each core's
    # per-window tile counts up to the max across cores.
    tiles_per_win = np.zeros((N_CORES, n_win), dtype=np.int64)
    for c, p in enumerate(plans):
        tw = p[2]
        tiles_per_win[c] = np.bincount(tw, minlength=n_win)
    # assign each core's windows to program slots in descending tile-count
    # order so the cross-core max per slot (the shared padded tile count)
    # is nearly tight; outputs are un-permuted on the host.
    orders = [np.argsort(-tiles_per_win[c], kind="stable")
              for c in range(N_CORES)]
    sorted_counts = np.stack([tiles_per_win[c][orders[c]]
                              for c in range(N_CORES)])
    common_tiles_w = sorted_counts.max(axis=0)
    T = int(common_tiles_w.sum())
    common_tile_window = np.repeat(np.arange(n_win), common_tiles_w)

    in_maps = []
    for c, p in enumerate(plans):
        gidx_c, segl_c, tw_c, inv_c, T_c, _ = p
        # re-pack this core's tiles into the common layout
        gidx_full = np.zeros((P, T), dtype=np.int32)
        segl_full = np.full((P, T), -1.0, dtype=np.float32)
        inv_full = inv_c[:, orders[c]].copy()
        src_pos = np.concatenate([[0], np.cumsum(tiles_per_win[c])])
        dst_pos = np.concatenate([[0], np.cumsum(common_tiles_w)])
        for j in range(n_win):
            w = orders[c][j]
            k = tiles_per_win[c][w]
            gidx_full[:, dst_pos[j]:dst_pos[j] + k] = \
                gidx_c[:, src_pos[w]:src_pos[w] + k]
            segl_full[:, dst_pos[j]:dst_pos[j] + k] = \
                segl_c[:, src_pos[w]:src_pos[w] + k]
        in_maps.append({
            "src": source,
            "gidx": gidx_full,
            "segl": segl_full,
            "invc": inv_full,
            "iota": iota_np,
        })

    key = (T, n_win, common_tile_window.tobytes())
    if key not in _compiled_cache:
        _compiled_cache[key] = _build_program(T, n_win, common_tile_window)
    nc = _compiled_cache[key]

    _last_state["nc"] = nc
    _last_state["in_maps"] = in_maps
    try:
        res = run_bass_kernel_spmd(nc, in_maps, core_ids=list(range(N_CORES)))
    except Exception:
        # transient NRT device wedge (NRT_EXEC_UNIT_UNRECOVERABLE) -- retry once
        res = run_bass_kernel_spmd(nc, in_maps, core_ids=list(range(N_CORES)))
    outs = []
    for c in range(N_CORES):
        o = res.results[c]["out"]          # [P, n_win slots, DIM]
        unperm = np.empty_like(o)
        unperm[:, orders[c], :] = o        # slot j holds window orders[c][j]
        outs.append(unperm.transpose(1, 0, 2).reshape(n_win * WIN, DIM)
                    [:SEGS_PER_CORE])
    return np.concatenate(outs, axis=0)


# state of the last kernel() invocation, for external profiling harnesses
_last_state = {}
